# revision 50
# baseline (speedup 1.0000x reference)
"""GCGRU cell (order-2 graph diffusion GRU) Trainium2 Bass kernel, v3.

Strategy: data-parallel over batch (B=16 -> 2 batches per core x 8 cores).
The whole adjacency matrix lives RESIDENT in SBUF as scaled fp8 (16.8 MB =
128 KiB/partition), loaded once instead of being streamed 4x in fp16. All
four diffusion stages (z1 = A z, z2 = A z1, zc1 = A rh, zc2 = A zc1) run as
fp8 DoubleRow matmuls (K=256 per pass, FD=512) in sz-form: the activation
tensor is PE-stationary (node-major, fp8, paired DR layout: node =
j2*256 + 2p + i so fp8 pairs sit in one 16-bit word) and adj is the wide
moving operand. Outputs are born channel-major; diffused conv feats (z1,
rh, zc1) are kept in fp8 with the scale folded into the host-side conv
weights, so matmuls read them directly (mixed fp8 moving x fp16 stationary
runs at the same rate). Node-major stationaries are produced by DMA XBAR
transposes of the fp8 data inside uint16 containers - no separate cast
step, half the transpose bytes, and only 2-4 large transposes per tensor
(each DMA transpose globally serializes the DMA rings, so fewer is faster).
Gate convs use fused [Wf|Wu] stationaries; candidate x/z2 conv terms are
pre-accumulated into c_x during the gate band loop; the final tanh/combine
is fused into stage D's group loop. PE transposes: zero.

Scales (static): adj x2^14 (below fp8 subnormal range otherwise), z x8,
z1 x128 / zc1 x128 at their fp8 casts (rh unscaled; its magnitude is ~0.5);
undone in the psum descale or host-folded into conv weights.
"""

import numpy as np
import ml_dtypes

import concourse.bass as bass
from concourse import bacc
import concourse.mybir as mybir
import concourse.tile as tile
from concourse.bass_utils import run_bass_kernel_spmd

# problem constants
B, D_IN, D_H, NN = 16, 32, 64, 4000
NCORES = 8
B_LOC = B // NCORES          # batches per core
C = D_IN + D_H               # 96 channels into each gate conv
BH = B_LOC * D_H             # stacked batch-hidden rows (128)
NP = 4096                    # node dim padded to a multiple of 512
CH = 128                     # node chunk
NCHUNK = NP // CH            # 32 node chunks
NJ2 = NCHUNK // 2            # 16 DoubleRow chunk pairs
BAND = 512                   # psum group band (m-range per diffusion group)
NG = NP // BAND              # 8 groups
NP8 = ml_dtypes.float8_e4m3  # TRN fp8_e4m3

F8 = mybir.dt.float8e4
F16 = mybir.dt.float16
F32 = mybir.dt.float32
U16 = mybir.dt.uint16
DR = mybir.MatmulPerfMode.DoubleRow

# static scales
S_A = float(2 ** 14)
S_Z = 8.0
S_Z1 = 128.0
S_ZC1 = 128.0
CA = S_Z1 / (S_A * S_Z)      # stage A psum -> z1 fp8 (scale S_Z1)
CB = 1.0 / (S_A * S_Z1)      # stage B psum -> z2 fp16 (true)
CC = S_ZC1 / S_A             # stage C psum -> zc1 fp8 (scale S_ZC1)
CD = 1.0 / (S_A * S_ZC1)     # stage D psum -> zc2 fp16 (true)


def build_program():
    nc = bacc.Bacc("TRN2", target_bir_lowering=False, debug=False)

    # ---- DRAM I/O (all host-prepped layouts, paired node order) ----
    # at8[p, ((g*NJ2+j2)*BAND + m)*2 + i] = S_A * adj[g*BAND+m, j2*256+2p+i]
    at_d = nc.dram_tensor("at", [CH, NG * NJ2 * 2 * BAND], F8,
                          kind="ExternalInput").ap()
    # ztdr[p, ((j2*B_LOC + b)*C + c)*2 + i] = S_Z * z[b, c, j2*256+2p+i]
    zt_d = nc.dram_tensor("zt", [CH, NCHUNK * B_LOC * C], F8,
                          kind="ExternalInput").ap()
    xh_d = nc.dram_tensor("xh", [B_LOC, C, NP], F16, kind="ExternalInput").ap()
    hbs_d = nc.dram_tensor("hbs", [BH, NP], F16, kind="ExternalInput").ap()
    # gate weights [Wf|Wu] fused per diffusion order (z1 col pre-divided)
    wg_d = nc.dram_tensor("wg", [3, C, 2 * D_H], F16, kind="ExternalInput").ap()
    # candidate weights: [x-order0; x-order2] stacked, and x-order1 (/S_Z1)
    wcxz_d = nc.dram_tensor("wcxz", [2 * D_IN, D_H], F16,
                            kind="ExternalInput").ap()
    wcx1_d = nc.dram_tensor("wcx1", [D_IN, D_H], F16,
                            kind="ExternalInput").ap()
    # candidate rh-part weights, batch-duplicated rows (k=1 pre-divided)
    wcrh_d = nc.dram_tensor("wcrh", [3, BH, D_H], F16, kind="ExternalInput").ap()
    bfu_d = nc.dram_tensor("bfu", [BH, 1], F32, kind="ExternalInput").ap()
    bc_d = nc.dram_tensor("bcb", [BH, 1], F32, kind="ExternalInput").ap()
    out_d = nc.dram_tensor("out", [B_LOC, D_H, NN], F16,
                           kind="ExternalOutput").ap()

    aps = dict(at_d=at_d, zt_d=zt_d, xh_d=xh_d, hbs_d=hbs_d, wg_d=wg_d,
               wcxz_d=wcxz_d, wcx1_d=wcx1_d, wcrh_d=wcrh_d, bfu_d=bfu_d,
               bc_d=bc_d, out_d=out_d)
    with tile.TileContext(nc) as tc:
        _body(tc, aps)
    nc.compile()
    return nc


def _body(tc, aps):
    nc = tc.nc
    SIG = mybir.ActivationFunctionType.Sigmoid
    TANH = mybir.ActivationFunctionType.Tanh
    COPY = mybir.ActivationFunctionType.Copy
    at_d, zt_d, xh_d, hbs_d = (aps[k] for k in ("at_d", "zt_d", "xh_d", "hbs_d"))
    out_d = aps["out_d"]

    with (
        tc.tile_pool(name="const", bufs=1) as cpool,     # persistent tiles
        tc.tile_pool(name="band", bufs=2) as bpool,      # rotating band tiles
        tc.tile_pool(name="psum", bufs=8, space="PSUM") as pspool,
    ):
        # ---- persistent loads ----
        at8 = cpool.tile([CH, NG * NJ2 * 2 * BAND], F8, tag="at8")
        GW = NJ2 * 2 * BAND  # columns per group slab
        JW = 2 * BAND        # columns per j2 unit
        ztdr = cpool.tile([CH, NCHUNK * B_LOC * C], F8, tag="ztdr")
        ZQ = NCHUNK * B_LOC * C // 4
        # head: slab-0 j2 0-5 fine-grained on sync, ztdr quarters first on
        # scalar followed by slab-0 j2 6-15; early slabs sync-biased 10/6
        # (the scalar ring starts behind by the ztdr + slab-0 back half)
        for q in range(3):
            nc.sync.dma_start(out=at8[:, q * 2 * JW:(q + 1) * 2 * JW],
                              in_=at_d[:, q * 2 * JW:(q + 1) * 2 * JW])
        for q in range(4):
            nc.scalar.dma_start(out=ztdr[:, q * ZQ:(q + 1) * ZQ],
                                in_=zt_d[:, q * ZQ:(q + 1) * ZQ])
        nc.scalar.dma_start(out=at8[:, 6 * JW:GW], in_=at_d[:, 6 * JW:GW])
        for g in range(1, NG):
            b0 = g * GW
            hw = (10 if g <= 3 else 8) * JW
            nc.sync.dma_start(out=at8[:, b0:b0 + hw], in_=at_d[:, b0:b0 + hw])
            nc.scalar.dma_start(out=at8[:, b0 + hw:b0 + GW],
                                in_=at_d[:, b0 + hw:b0 + GW])

        wg_sb = [cpool.tile([C, 2 * D_H], F16, tag=f"wg{k}", name=f"wg{k}")
                 for k in range(3)]
        wcxz_sb = cpool.tile([2 * D_IN, D_H], F16, tag="wcxz")
        nc.scalar.dma_start(out=wcxz_sb[:], in_=aps["wcxz_d"][:])
        wcx1_sb = cpool.tile([D_IN, D_H], F16, tag="wcx1")
        nc.scalar.dma_start(out=wcx1_sb[:], in_=aps["wcx1_d"][:])
        wcrh_sb = [cpool.tile([BH, D_H], F16, tag=f"wcrh{k}", name=f"wcrh{k}")
                   for k in range(3)]
        for k in range(3):
            nc.scalar.dma_start(out=wg_sb[k][:], in_=aps["wg_d"][k])
            nc.scalar.dma_start(out=wcrh_sb[k][:], in_=aps["wcrh_d"][k])
        bfu_sb = cpool.tile([BH, 1], F32, tag="bfu")
        nc.scalar.dma_start(out=bfu_sb[:], in_=aps["bfu_d"][:])
        bc_sb = cpool.tile([BH, 1], F32, tag="bc")
        nc.scalar.dma_start(out=bc_sb[:], in_=aps["bc_d"][:])

        # persistent activation tensors. z1f8/rhf8 columns are b-outer
        # (b*NP + node) so per-batch half transposes are contiguous.
        z1f8 = cpool.tile([C, B_LOC * NP], F8, tag="z1f8")
        rhf8 = cpool.tile([D_H, B_LOC * NP], F8, tag="rhf8")
        z1dr = cpool.tile([CH, NCHUNK * B_LOC * C], F8, tag="z1dr")
        rhdr = cpool.tile([CH, NCHUNK * BH], F8, tag="rhdr")
        zc1dr = cpool.tile([CH, NCHUNK * BH], F8, tag="zc1dr")
        u_st = cpool.tile([BH, NP], F16, tag="u_st")
        c_x = cpool.tile([BH, NP], F16, tag="c_x")

        def z1sl(g, b):
            return z1f8[:, b * NP + g * BAND:b * NP + (g + 1) * BAND]

        def rhsl(g, b):
            return rhf8[:, b * NP + g * BAND:b * NP + (g + 1) * BAND]

        # paired node mapping (node = j2*256 + 2p + i) stored i-major: the
        # DR LDWEIGHTS ISA requires the pair-dim step to be 16-aligned
        def adj_mv(g, j2):
            base = (g * NJ2 + j2) * 2 * BAND
            return at8[:, base:base + 2 * BAND].rearrange(
                "p (i m) -> p i m", i=2)

        def zt_st(src, j2, b):
            base = j2 * 2 * B_LOC * C
            return src[:, base:base + 2 * B_LOC * C].rearrange(
                "p (i bc) -> p i bc", i=2)[:, :, b * C:(b + 1) * C]

        def bh_st(src, j2):
            base = j2 * 2 * BH
            return src[:, base:base + 2 * BH].rearrange(
                "p (i c) -> p i c", i=2)

        # ---- diffusion group helpers (all DoubleRow fp8, FD=512) ----
        def diff_gates(src, g, name):
            pss = []
            for b in range(B_LOC):
                ps = pspool.tile([C, BAND], F32, tag="ps", name=f"{name}{b}")
                for j2 in range(NJ2):
                    nc.tensor.matmul(ps[:, :], lhsT=zt_st(src, j2, b),
                                     rhs=adj_mv(g, j2), start=(j2 == 0),
                                     stop=(j2 == NJ2 - 1), perf_mode=DR)
                pss.append(ps)
            return pss

        def diff_cand(src, g, name):
            ps = pspool.tile([BH, BAND], F32, tag="ps", name=name)
            for j2 in range(NJ2):
                nc.tensor.matmul(ps[:, :], lhsT=bh_st(src, j2),
                                 rhs=adj_mv(g, j2), start=(j2 == 0),
                                 stop=(j2 == NJ2 - 1), perf_mode=DR)
            return ps

        def nm_half(srcf8, dst, b, h, p, cw):
            """Transpose fp8 half-tensor (bands 4h..4h+3, batch b) into the
            paired node-major DR layout: one u16-container DMA transpose to
            a temp (pairs byte-adjacent), then a DVE de-interleave into the
            i-major dst columns (j2, i, bc)."""
            c = cw // B_LOC
            in_ap = srcf8[:, b * NP + h * 2048:b * NP + (h + 1) * 2048]
            nm8 = bpool.tile([CH, 1536], F8, tag="nm8", name="nm8")
            nc.sync.dma_start(
                out=nm8[:, 0:16 * c].bitcast(U16).rearrange(
                    "q (j c) -> q j c", j=8),
                in_=in_ap.bitcast(U16), transpose=True)
            dst_ap = dst[:, h * 8 * 2 * cw:(h + 1) * 8 * 2 * cw].rearrange(
                "q (j i bc) -> q j i bc", j=8, i=2)[:, :, :,
                                                   b * c:(b + 1) * c]
            src_ap = nm8[:, 0:16 * c].rearrange("q (j c i) -> q j i c",
                                                j=8, i=2)
            nc.vector.tensor_copy(out=dst_ap, in_=src_ap)

        # ================= stage B band loads (prefetched) ===============
        def loads(g):
            sl = slice(g * BAND, (g + 1) * BAND)
            xh_b = [bpool.tile([C, BAND], F16, tag=f"xhb{b}", name=f"xhb{b}", bufs=3)
                    for b in range(B_LOC)]
            h_b = [bpool.tile([D_H, BAND], F16, tag=f"hb{b}", name=f"hb{b}", bufs=3)
                   for b in range(B_LOC)]
            for b in range(B_LOC):
                nc.scalar.dma_start(out=xh_b[b][:, :], in_=xh_d[b][:, sl])
                nc.scalar.dma_start(out=h_b[b][:, :],
                                    in_=hbs_d[b * D_H:(b + 1) * D_H, sl])
            return xh_b, h_b

        lds = [loads(0), loads(1)]

        # ================= stage A: z1 = A z =================
        for g in range(NG):
            pss = diff_gates(ztdr, g, "psa")
            for b in range(B_LOC):
                nc.vector.tensor_scalar_mul(z1sl(g, b), pss[b][:, :], CA)
            if g == 3 or g == NG - 1:
                for b in range(B_LOC):
                    nm_half(z1f8, z1dr, b, g // 4, C, B_LOC * C)

        # ========== stage B: z2 = A z1, fused gate convs ==========
        def conv_band(g, xh_b, h_b):
            sl = slice(g * BAND, (g + 1) * BAND)
            z2_b = z2bands[g % 2]
            # fused [Wf|Wu] stationaries: psum rows 0:64 = f, 64:128 = u
            psfu = [pspool.tile([BH, BAND], F32, tag="ps", name=f"psfu{b}")
                    for b in range(B_LOC)]
            pcx = pspool.tile([BH, BAND], F32, tag="ps", name="pcx")
            for b in range(B_LOC):
                # order matters: z2-dependent matmul last (z2 copy overlaps)
                nc.tensor.matmul(psfu[b][:, :], lhsT=wg_sb[0],
                                 rhs=xh_b[b][:, :], start=True, stop=False)
                nc.tensor.matmul(psfu[b][:, :], lhsT=wg_sb[1],
                                 rhs=z1sl(g, b), start=False, stop=False)
                nc.tensor.matmul(psfu[b][:, :], lhsT=wg_sb[2],
                                 rhs=z2_b[b][:, :], start=False, stop=True)
            sig = [bpool.tile([BH, BAND], F16, tag="sig", name=f"sig{b}")
                   for b in range(B_LOC)]
            # candidate x-feats [x; z2] concatenated on partitions
            xcat = bpool.tile([2 * D_IN, 2 * BAND], F16, tag="xcat",
                              name="xcat")
            for b in range(B_LOC):
                nc.scalar.activation(sig[b][:, :], psfu[b][:, :], SIG,
                                     bias=bfu_sb[:, :])
                nc.vector.tensor_mul(out=rhsl(g, b), in0=sig[b][0:D_H, :],
                                     in1=h_b[b][:, :])
                cs = slice(b * BAND, (b + 1) * BAND)
                nc.vector.tensor_copy(out=xcat[0:D_IN, cs],
                                      in_=xh_b[b][0:D_IN, :])
                nc.vector.tensor_copy(out=xcat[D_IN:2 * D_IN, cs],
                                      in_=z2_b[b][0:D_IN, :])
            # u rows (64:128 of sig) into b-stacked u_st; b0 needs a
            # cross-quadrant move -> two 32-wide DVE copies
            nc.vector.tensor_copy(out=u_st[0:32, sl], in_=sig[0][64:96, :])
            nc.vector.tensor_copy(out=u_st[32:64, sl], in_=sig[0][96:128, :])
            nc.vector.tensor_copy(out=u_st[64:128, sl], in_=sig[1][64:128, :])
            for b in range(B_LOC):
                rows = slice(b * D_H, (b + 1) * D_H)
                cs = slice(b * BAND, (b + 1) * BAND)
                nc.tensor.matmul(pcx[rows, :], lhsT=wcxz_sb[:, :],
                                 rhs=xcat[:, cs], start=True, stop=False)
                nc.tensor.matmul(pcx[rows, :], lhsT=wcx1_sb[:, :],
                                 rhs=z1sl(g, b)[0:D_IN, :], start=False,
                                 stop=False)
                nc.tensor.matmul(pcx[rows, :], lhsT=wcrh_sb[0][0:D_H, :],
                                 rhs=rhsl(g, b), start=False, stop=True)
            nc.scalar.activation(c_x[:, sl], pcx[:, :], COPY)
            if g == 3 or g == NG - 1:
                for b in range(B_LOC):
                    nm_half(rhf8, rhdr, b, g // 4, D_H, BH)

        # stage-B diffusion groups split front (j2 0..7, z1-half-1 only) /
        # back (j2 8..13) / tail (j2 14,15 + psum copies): three fronts of
        # independent work queue ahead of the first half-2-dependent matmul
        # so the z1 half-2 transpose chain never stalls the in-order PE.
        def diffB_front(g):
            pss = [pspool.tile([C, BAND], F32, tag="ps", name=f"psb{b}")
                   for b in range(B_LOC)]
            for j2 in range(NJ2 // 2):
                for b in range(B_LOC):
                    nc.tensor.matmul(pss[b][:, :], lhsT=zt_st(z1dr, j2, b),
                                     rhs=adj_mv(g, j2), start=(j2 == 0),
                                     stop=False, perf_mode=DR)
            return pss

        def diffB_back(g, pss):
            for j2 in range(NJ2 // 2, NJ2 - 2):
                for b in range(B_LOC):
                    nc.tensor.matmul(pss[b][:, :], lhsT=zt_st(z1dr, j2, b),
                                     rhs=adj_mv(g, j2), start=False,
                                     stop=False, perf_mode=DR)

        def diffB_tail(g, pss):
            z2_b = [bpool.tile([C, BAND], F16, tag=f"z2b{b}", name=f"z2b{b}")
                    for b in range(B_LOC)]
            for b in range(B_LOC):
                for j2 in (NJ2 - 2, NJ2 - 1):
                    nc.tensor.matmul(pss[b][:, :], lhsT=zt_st(z1dr, j2, b),
                                     rhs=adj_mv(g, j2), start=False,
                                     stop=(j2 == NJ2 - 1), perf_mode=DR)
                nc.scalar.activation(z2_b[b][:, :], pss[b][:, :], COPY,
                                     scale=CB)
            return z2_b

        z2bands = [None, None]
        heads = [diffB_front(0), diffB_front(1), diffB_front(2)]
        diffB_back(0, heads[0])
        z2bands[0] = diffB_tail(0, heads[0])
        for g in range(NG):
            if g + 1 < NG:
                diffB_back(g + 1, heads[g + 1])
                z2bands[(g + 1) % 2] = diffB_tail(g + 1, heads[g + 1])
            if g + 2 < NG:
                lds.append(loads(g + 2))
            conv_band(g, *lds[g])
            if g + 3 < NG:
                heads.append(diffB_front(g + 3))

        # ================= stage C: zc1 = A rh ==================
        def consC(g, ps):
            sl = slice(g * BAND, (g + 1) * BAND)
            zc1_b = bpool.tile([BH, BAND], F8, tag="zc1f8", name="zc1b")
            nc.vector.tensor_scalar_mul(zc1_b[:, :], ps[:, :], CC)
            pc1 = pspool.tile([BH, BAND], F32, tag="ps", name="pc1")
            for b in range(B_LOC):
                rows = slice(b * D_H, (b + 1) * D_H)
                nc.tensor.matmul(pc1[rows, :], lhsT=wcrh_sb[1][rows, :],
                                 rhs=zc1_b[rows, :], start=True, stop=True)
            nc.vector.tensor_add(out=c_x[:, sl], in0=c_x[:, sl],
                                 in1=pc1[:, :])
            # one u16 transpose per band + DVE de-interleave into (j2, i, c)
            nm8 = bpool.tile([CH, 1536], F8, tag="nm8", name="nm8")
            nc.sync.dma_start(
                out=nm8[:, 0:512].bitcast(U16).rearrange(
                    "q (j c) -> q j c", j=2),
                in_=zc1_b[:, :].bitcast(U16), transpose=True)
            nc.vector.tensor_copy(
                out=zc1dr[:, 2 * g * 2 * CH:(2 * g + 2) * 2 * CH].rearrange(
                    "q (j i c) -> q j i c", j=2, i=2),
                in_=nm8[:, 0:512].rearrange("q (j c i) -> q j i c",
                                            j=2, i=2))

        psC = diff_cand(rhdr, 0, "psc")
        for g in range(NG):
            psN = diff_cand(rhdr, g + 1, "psc") if g + 1 < NG else None
            consC(g, psC)
            psC = psN

        # ====== stage D: zc2 = A zc1, fused tanh + GRU combine ======
        def consD(g, ps):
            sl = slice(g * BAND, (g + 1) * BAND)
            hD = bpool.tile([BH, BAND], F16, tag="xhb0", name="hD", bufs=3)
            nc.sync.dma_start(out=hD[:, :], in_=hbs_d[:, sl])
            zc2_b = bpool.tile([BH, BAND], F16, tag="xhb1", name="zc2b", bufs=3)
            nc.scalar.activation(zc2_b[:, :], ps[:, :], COPY, scale=CD)
            pc2 = pspool.tile([BH, BAND], F32, tag="ps", name="pc2")
            cst = bpool.tile([BH, BAND], F16, tag="z2b0", name="cst")
            # the last band runs the combine in two halves so its final
            # store chain is half as long (this band gates kernel teardown)
            nhal = 2 if g == NG - 1 else 1
            hwd = BAND // nhal
            for hh in range(nhal):
                cs = slice(hh * hwd, (hh + 1) * hwd)
                slh = slice(g * BAND + hh * hwd, g * BAND + (hh + 1) * hwd)
                for b in range(B_LOC):
                    rows = slice(b * D_H, (b + 1) * D_H)
                    nc.tensor.matmul(pc2[rows, cs], lhsT=wcrh_sb[2][rows, :],
                                     rhs=zc2_b[rows, cs], start=True,
                                     stop=True)
                nc.vector.tensor_add(out=pc2[:, cs], in0=pc2[:, cs],
                                     in1=c_x[:, slh])
                nc.scalar.activation(cst[:, cs], pc2[:, cs], TANH,
                                     bias=bc_sb[:, :])
                nc.vector.tensor_sub(out=hD[:, cs], in0=hD[:, cs],
                                     in1=cst[:, cs])
                nc.vector.tensor_mul(out=hD[:, cs], in0=u_st[:, slh],
                                     in1=hD[:, cs])
                nc.vector.tensor_add(out=cst[:, cs], in0=cst[:, cs],
                                     in1=hD[:, cs])
                w = min((hh + 1) * hwd, NN - g * BAND)
                if w > hh * hwd:
                    for b in range(B_LOC):
                        nc.sync.dma_start(
                            out=out_d[b][:, g * BAND + hh * hwd:
                                         g * BAND + w],
                            in_=cst[b * D_H:(b + 1) * D_H, hh * hwd:w])

        psD = diff_cand(zc1dr, 0, "psd")
        for g in range(NG):
            psN = diff_cand(zc1dr, g + 1, "psd") if g + 1 < NG else None
            consD(g, psD)
            psD = psN


# ---- host-side driver ----
_CACHED_NC = None
TRACE = False
TRACE_DIR = None
LAST_RESULTS = None


def _host_prep(x, h, adj, Wf, bf, Wu, bu, Wc, bc):
    A = np.zeros((NP, NP), dtype=np.float32)
    A[:NN, :NN] = adj * S_A
    # paired node order, i-major: at8[p, (g, j2, i, m)] = A[g*512+m,
    # j2*256+2p+i]
    at8 = np.ascontiguousarray(
        A.reshape(NG, BAND, NJ2, CH, 2).transpose(3, 0, 2, 4, 1)
        .reshape(CH, -1)).astype(NP8)

    WfT = Wf.T.astype(np.float32)     # [288, 64]
    WuT = Wu.T.astype(np.float32)
    WcT = Wc.T.astype(np.float32)
    wgk = []
    for k in range(3):
        m = np.concatenate([WfT[k * 96:(k + 1) * 96],
                            WuT[k * 96:(k + 1) * 96]], axis=1)
        if k == 1:
            m = m / S_Z1
        wgk.append(m)
    wg = np.ascontiguousarray(np.stack(wgk)).astype(np.float16)
    wcxz = np.ascontiguousarray(
        np.concatenate([WcT[0:32], WcT[192:224]], axis=0)).astype(np.float16)
    wcx1 = np.ascontiguousarray(WcT[96:128] / S_Z1).astype(np.float16)
    wcrh = np.ascontiguousarray(np.stack(
        [np.concatenate([WcT[32:96]] * B_LOC, axis=0),
         np.concatenate([WcT[128:192]] * B_LOC, axis=0) / S_ZC1,
         np.concatenate([WcT[224:288]] * B_LOC, axis=0)])).astype(np.float16)

    bfu = np.concatenate([bf, bu]).reshape(BH, 1).astype(np.float32)
    bc2 = np.concatenate([bc] * B_LOC).reshape(BH, 1).astype(np.float32)
    shared = {"at": at8, "wg": wg, "wcxz": wcxz, "wcx1": wcx1, "wcrh": wcrh,
              "bfu": bfu, "bcb": bc2}

    in_maps = []
    for core in range(NCORES):
        bs = slice(core * B_LOC, (core + 1) * B_LOC)
        zp = np.zeros((B_LOC, C, NP), dtype=np.float32)
        zp[:, :D_IN, :NN] = x[bs]
        zp[:, D_IN:, :NN] = h[bs]
        # paired, i-major: ztdr[p, (j2, i, b, c)] = S_Z * z[b, c, j2*256+2p+i]
        ztdr = np.ascontiguousarray(
            (zp.reshape(B_LOC, C, NJ2, CH, 2).transpose(3, 2, 4, 0, 1)
             .reshape(CH, -1)) * S_Z).astype(NP8)
        hbs = np.ascontiguousarray(
            zp[:, D_IN:, :].reshape(BH, NP)).astype(np.float16)
        in_maps.append(dict(shared, zt=ztdr, xh=zp.astype(np.float16),
                            hbs=hbs))
    return in_maps


def kernel(**inputs):
    global _CACHED_NC, LAST_RESULTS
    inputs = {k: np.asarray(v) for k, v in inputs.items()}
    if _CACHED_NC is None:
        _CACHED_NC = build_program()
    in_maps = _host_prep(**inputs)
    kw = {}
    if TRACE:
        kw = dict(trace=True, tmpdir=TRACE_DIR)
    res = run_bass_kernel_spmd(_CACHED_NC, in_maps,
                               core_ids=list(range(NCORES)), **kw)
    LAST_RESULTS = res
    outs = [res.results[i]["out"] for i in range(NCORES)]
    return np.concatenate(outs, axis=0).astype(np.float32)


if __name__ == "__main__":
    rng = np.random.default_rng(0)
    ins = {
        "x": rng.standard_normal((B, D_IN, NN), dtype=np.float32),
        "h": rng.standard_normal((B, D_H, NN), dtype=np.float32),
        "adj": rng.random((NN, NN), dtype=np.float32) / NN,
        "Wf": rng.standard_normal((D_H, 3 * C), dtype=np.float32) * 0.05,
        "Wu": rng.standard_normal((D_H, 3 * C), dtype=np.float32) * 0.05,
        "Wc": rng.standard_normal((D_H, 3 * C), dtype=np.float32) * 0.05,
        "bf": rng.standard_normal(D_H).astype(np.float32) * 0.05,
        "bu": rng.standard_normal(D_H).astype(np.float32) * 0.05,
        "bc": rng.standard_normal(D_H).astype(np.float32) * 0.05,
    }
    out = kernel(**ins)
    print(out.shape, out.dtype)


# revision 51
# speedup vs baseline: 1.0323x; 1.0323x over previous
"""GCGRU cell (order-2 graph diffusion GRU) Trainium2 Bass kernel, v3.

Strategy: data-parallel over batch (B=16 -> 2 batches per core x 8 cores).
The whole adjacency matrix lives RESIDENT in SBUF as scaled fp8 (16.8 MB =
128 KiB/partition), loaded once instead of being streamed 4x in fp16. All
four diffusion stages (z1 = A z, z2 = A z1, zc1 = A rh, zc2 = A zc1) run as
fp8 DoubleRow matmuls (K=256 per pass, FD=512) in sz-form: the activation
tensor is PE-stationary (node-major, fp8, paired DR layout: node =
j2*256 + 2p + i so fp8 pairs sit in one 16-bit word) and adj is the wide
moving operand. Outputs are born channel-major; diffused conv feats (z1,
rh, zc1) are kept in fp8 with the scale folded into the host-side conv
weights, so matmuls read them directly (mixed fp8 moving x fp16 stationary
runs at the same rate). Node-major stationaries are produced by DMA XBAR
transposes of the fp8 data inside uint16 containers - no separate cast
step, half the transpose bytes, and only 2-4 large transposes per tensor
(each DMA transpose globally serializes the DMA rings, so fewer is faster).
Gate convs use fused [Wf|Wu] stationaries; candidate x/z2 conv terms are
pre-accumulated into c_x during the gate band loop; the final tanh/combine
is fused into stage D's group loop. PE transposes: zero.

Scales (static): adj x2^14 (below fp8 subnormal range otherwise), z x8,
z1 x128 / zc1 x128 at their fp8 casts (rh unscaled; its magnitude is ~0.5);
undone in the psum descale or host-folded into conv weights.
"""

import numpy as np
import ml_dtypes

import concourse.bass as bass
from concourse import bacc
import concourse.mybir as mybir
import concourse.tile as tile
from concourse.bass_utils import run_bass_kernel_spmd

# problem constants
B, D_IN, D_H, NN = 16, 32, 64, 4000
NCORES = 8
B_LOC = B // NCORES          # batches per core
C = D_IN + D_H               # 96 channels into each gate conv
BH = B_LOC * D_H             # stacked batch-hidden rows (128)
NP = 4096                    # node dim padded to a multiple of 512
CH = 128                     # node chunk
NCHUNK = NP // CH            # 32 node chunks
NJ2 = NCHUNK // 2            # 16 DoubleRow chunk pairs
BAND = 512                   # psum group band (m-range per diffusion group)
NG = NP // BAND              # 8 groups
NP8 = ml_dtypes.float8_e4m3  # TRN fp8_e4m3

F8 = mybir.dt.float8e4
F16 = mybir.dt.float16
F32 = mybir.dt.float32
U16 = mybir.dt.uint16
DR = mybir.MatmulPerfMode.DoubleRow

# static scales
S_A = float(2 ** 14)
S_Z = 8.0
S_Z1 = 128.0
S_ZC1 = 128.0
CA = S_Z1 / (S_A * S_Z)      # stage A psum -> z1 fp8 (scale S_Z1)
CB = 1.0 / (S_A * S_Z1)      # stage B psum -> z2 fp16 (true)
CC = S_ZC1 / S_A             # stage C psum -> zc1 fp8 (scale S_ZC1)
CD = 1.0 / (S_A * S_ZC1)     # stage D psum -> zc2 fp16 (true)


def build_program():
    nc = bacc.Bacc("TRN2", target_bir_lowering=False, debug=False)

    # ---- DRAM I/O (all host-prepped layouts, paired node order) ----
    # at8[p, ((g*NJ2+j2)*BAND + m)*2 + i] = S_A * adj[g*BAND+m, j2*256+2p+i]
    at_d = nc.dram_tensor("at", [CH, NG * NJ2 * 2 * BAND], F8,
                          kind="ExternalInput").ap()
    # ztdr[p, ((j2*B_LOC + b)*C + c)*2 + i] = S_Z * z[b, c, j2*256+2p+i]
    zt_d = nc.dram_tensor("zt", [CH, NCHUNK * B_LOC * C], F8,
                          kind="ExternalInput").ap()
    xh_d = nc.dram_tensor("xh", [B_LOC, C, NP], F16, kind="ExternalInput").ap()
    hbs_d = nc.dram_tensor("hbs", [BH, NP], F16, kind="ExternalInput").ap()
    # gate weights [Wf|Wu] fused per diffusion order (z1 col pre-divided)
    wg_d = nc.dram_tensor("wg", [3, C, 2 * D_H], F16, kind="ExternalInput").ap()
    # candidate weights: [x-order0; x-order2] stacked, and x-order1 (/S_Z1)
    wcxz_d = nc.dram_tensor("wcxz", [2 * D_IN, D_H], F16,
                            kind="ExternalInput").ap()
    wcx1_d = nc.dram_tensor("wcx1", [D_IN, D_H], F16,
                            kind="ExternalInput").ap()
    # candidate rh-part weights, batch-duplicated rows (k=1 pre-divided)
    wcrh_d = nc.dram_tensor("wcrh", [3, BH, D_H], F16, kind="ExternalInput").ap()
    bfu_d = nc.dram_tensor("bfu", [BH, 1], F32, kind="ExternalInput").ap()
    bc_d = nc.dram_tensor("bcb", [BH, 1], F32, kind="ExternalInput").ap()
    out_d = nc.dram_tensor("out", [B_LOC, D_H, NN], F16,
                           kind="ExternalOutput").ap()

    aps = dict(at_d=at_d, zt_d=zt_d, xh_d=xh_d, hbs_d=hbs_d, wg_d=wg_d,
               wcxz_d=wcxz_d, wcx1_d=wcx1_d, wcrh_d=wcrh_d, bfu_d=bfu_d,
               bc_d=bc_d, out_d=out_d)
    with tile.TileContext(nc) as tc:
        _body(tc, aps)
    nc.compile()
    return nc


def _body(tc, aps):
    nc = tc.nc
    SIG = mybir.ActivationFunctionType.Sigmoid
    TANH = mybir.ActivationFunctionType.Tanh
    COPY = mybir.ActivationFunctionType.Copy
    at_d, zt_d, xh_d, hbs_d = (aps[k] for k in ("at_d", "zt_d", "xh_d", "hbs_d"))
    out_d = aps["out_d"]

    with (
        tc.tile_pool(name="const", bufs=1) as cpool,     # persistent tiles
        tc.tile_pool(name="band", bufs=2) as bpool,      # rotating band tiles
        tc.tile_pool(name="psum", bufs=8, space="PSUM") as pspool,
    ):
        # ---- persistent loads ----
        at8 = cpool.tile([CH, NG * NJ2 * 2 * BAND], F8, tag="at8")
        GW = NJ2 * 2 * BAND  # columns per group slab
        JW = 2 * BAND        # columns per j2 unit
        ztdr = cpool.tile([CH, NCHUNK * B_LOC * C], F8, tag="ztdr")
        ZQ = NCHUNK * B_LOC * C // 4
        # head: slab-0 j2 0-5 fine-grained on sync, ztdr quarters first on
        # scalar followed by slab-0 j2 6-15; early slabs sync-biased 10/6
        # (the scalar ring starts behind by the ztdr + slab-0 back half)
        for q in range(3):
            nc.sync.dma_start(out=at8[:, q * 2 * JW:(q + 1) * 2 * JW],
                              in_=at_d[:, q * 2 * JW:(q + 1) * 2 * JW])
        for q in range(4):
            nc.scalar.dma_start(out=ztdr[:, q * ZQ:(q + 1) * ZQ],
                                in_=zt_d[:, q * ZQ:(q + 1) * ZQ])
        nc.scalar.dma_start(out=at8[:, 6 * JW:GW], in_=at_d[:, 6 * JW:GW])
        for g in range(1, NG):
            b0 = g * GW
            hw = (10 if g <= 3 else 8) * JW
            nc.sync.dma_start(out=at8[:, b0:b0 + hw], in_=at_d[:, b0:b0 + hw])
            nc.scalar.dma_start(out=at8[:, b0 + hw:b0 + GW],
                                in_=at_d[:, b0 + hw:b0 + GW])

        wg_sb = [cpool.tile([C, 2 * D_H], F16, tag=f"wg{k}", name=f"wg{k}")
                 for k in range(3)]
        wcxz_sb = cpool.tile([2 * D_IN, D_H], F16, tag="wcxz")
        nc.scalar.dma_start(out=wcxz_sb[:], in_=aps["wcxz_d"][:])
        wcx1_sb = cpool.tile([D_IN, D_H], F16, tag="wcx1")
        nc.scalar.dma_start(out=wcx1_sb[:], in_=aps["wcx1_d"][:])
        wcrh_sb = [cpool.tile([BH, D_H], F16, tag=f"wcrh{k}", name=f"wcrh{k}")
                   for k in range(3)]
        for k in range(3):
            nc.scalar.dma_start(out=wg_sb[k][:], in_=aps["wg_d"][k])
            nc.scalar.dma_start(out=wcrh_sb[k][:], in_=aps["wcrh_d"][k])
        bfu_sb = cpool.tile([BH, 1], F32, tag="bfu")
        nc.scalar.dma_start(out=bfu_sb[:], in_=aps["bfu_d"][:])
        bc_sb = cpool.tile([BH, 1], F32, tag="bc")
        nc.scalar.dma_start(out=bc_sb[:], in_=aps["bc_d"][:])

        # persistent activation tensors. z1f8/rhf8 columns are b-outer
        # (b*NP + node) so per-batch half transposes are contiguous.
        z1f8 = cpool.tile([C, B_LOC * NP], F8, tag="z1f8")
        rhf8 = cpool.tile([D_H, B_LOC * NP], F8, tag="rhf8")
        z1dr = cpool.tile([CH, NCHUNK * B_LOC * C], F8, tag="z1dr")
        rhdr = cpool.tile([CH, NCHUNK * BH], F8, tag="rhdr")
        zc1dr = cpool.tile([CH, NCHUNK * BH], F8, tag="zc1dr")
        u_st = cpool.tile([BH, NP], F16, tag="u_st")
        c_x = cpool.tile([BH, NP], F16, tag="c_x")

        def z1sl(g, b):
            return z1f8[:, b * NP + g * BAND:b * NP + (g + 1) * BAND]

        def rhsl(g, b):
            return rhf8[:, b * NP + g * BAND:b * NP + (g + 1) * BAND]

        # paired node mapping (node = j2*256 + 2p + i) stored i-major: the
        # DR LDWEIGHTS ISA requires the pair-dim step to be 16-aligned
        def adj_mv(g, j2):
            base = (g * NJ2 + j2) * 2 * BAND
            return at8[:, base:base + 2 * BAND].rearrange(
                "p (i m) -> p i m", i=2)

        def zt_st(src, j2, b):
            base = j2 * 2 * B_LOC * C
            return src[:, base:base + 2 * B_LOC * C].rearrange(
                "p (i bc) -> p i bc", i=2)[:, :, b * C:(b + 1) * C]

        def bh_st(src, j2):
            base = j2 * 2 * BH
            return src[:, base:base + 2 * BH].rearrange(
                "p (i c) -> p i c", i=2)

        # ---- diffusion group helpers (all DoubleRow fp8, FD=512) ----
        def diff_gates(src, g, name):
            pss = []
            for b in range(B_LOC):
                ps = pspool.tile([C, BAND], F32, tag="ps", name=f"{name}{b}")
                for j2 in range(NJ2):
                    nc.tensor.matmul(ps[:, :], lhsT=zt_st(src, j2, b),
                                     rhs=adj_mv(g, j2), start=(j2 == 0),
                                     stop=(j2 == NJ2 - 1), perf_mode=DR)
                pss.append(ps)
            return pss

        def diff_cand(src, g, name):
            ps = pspool.tile([BH, BAND], F32, tag="ps", name=name)
            for j2 in range(NJ2):
                nc.tensor.matmul(ps[:, :], lhsT=bh_st(src, j2),
                                 rhs=adj_mv(g, j2), start=(j2 == 0),
                                 stop=(j2 == NJ2 - 1), perf_mode=DR)
            return ps

        def nm_half(srcf8, dst, b, h, p, cw):
            """Transpose fp8 half-tensor (bands 4h..4h+3, batch b) into the
            paired node-major DR layout: one u16-container DMA transpose to
            a temp (pairs byte-adjacent), then a DVE de-interleave into the
            i-major dst columns (j2, i, bc)."""
            c = cw // B_LOC
            in_ap = srcf8[:, b * NP + h * 2048:b * NP + (h + 1) * 2048]
            nm8 = bpool.tile([CH, 1536], F8, tag="nm8", name="nm8")
            nc.sync.dma_start(
                out=nm8[:, 0:16 * c].bitcast(U16).rearrange(
                    "q (j c) -> q j c", j=8),
                in_=in_ap.bitcast(U16), transpose=True)
            dst_ap = dst[:, h * 8 * 2 * cw:(h + 1) * 8 * 2 * cw].rearrange(
                "q (j i bc) -> q j i bc", j=8, i=2)[:, :, :,
                                                   b * c:(b + 1) * c]
            src_ap = nm8[:, 0:16 * c].rearrange("q (j c i) -> q j i c",
                                                j=8, i=2)
            nc.vector.tensor_copy(out=dst_ap, in_=src_ap)

        # ================= stage B band loads (prefetched) ===============
        def loads(g):
            sl = slice(g * BAND, (g + 1) * BAND)
            xh_b = [bpool.tile([C, BAND], F16, tag=f"xhb{b}", name=f"xhb{b}")
                    for b in range(B_LOC)]
            h_b = [bpool.tile([D_H, BAND], F16, tag=f"hb{b}", name=f"hb{b}")
                   for b in range(B_LOC)]
            for b in range(B_LOC):
                nc.scalar.dma_start(out=xh_b[b][:, :], in_=xh_d[b][:, sl])
                nc.scalar.dma_start(out=h_b[b][:, :],
                                    in_=hbs_d[b * D_H:(b + 1) * D_H, sl])
            return xh_b, h_b

        lds = [loads(0), loads(1)]

        # ================= stage A: z1 = A z =================
        for g in range(NG):
            pss = diff_gates(ztdr, g, "psa")
            for b in range(B_LOC):
                nc.vector.tensor_scalar_mul(z1sl(g, b), pss[b][:, :], CA)
            if g == 3 or g == NG - 1:
                for b in range(B_LOC):
                    nm_half(z1f8, z1dr, b, g // 4, C, B_LOC * C)

        # ========== stage B: z2 = A z1, fused gate convs ==========
        def conv_band(g, xh_b, h_b):
            sl = slice(g * BAND, (g + 1) * BAND)
            z2_b = z2bands[g % 2]
            # fused [Wf|Wu] stationaries: psum rows 0:64 = f, 64:128 = u
            psfu = [pspool.tile([BH, BAND], F32, tag="ps", name=f"psfu{b}")
                    for b in range(B_LOC)]
            pcx = pspool.tile([BH, BAND], F32, tag="ps", name="pcx")
            for b in range(B_LOC):
                # order matters: z2-dependent matmul last (z2 copy overlaps)
                nc.tensor.matmul(psfu[b][:, :], lhsT=wg_sb[0],
                                 rhs=xh_b[b][:, :], start=True, stop=False)
                nc.tensor.matmul(psfu[b][:, :], lhsT=wg_sb[1],
                                 rhs=z1sl(g, b), start=False, stop=False)
                nc.tensor.matmul(psfu[b][:, :], lhsT=wg_sb[2],
                                 rhs=z2_b[b][:, :], start=False, stop=True)
            sig = [bpool.tile([BH, BAND], F16, tag="sig", name=f"sig{b}")
                   for b in range(B_LOC)]
            # candidate x-feats [x; z2] concatenated on partitions
            xcat = bpool.tile([2 * D_IN, 2 * BAND], F16, tag="xcat",
                              name="xcat")
            for b in range(B_LOC):
                nc.scalar.activation(sig[b][:, :], psfu[b][:, :], SIG,
                                     bias=bfu_sb[:, :])
                nc.vector.tensor_mul(out=rhsl(g, b), in0=sig[b][0:D_H, :],
                                     in1=h_b[b][:, :])
                cs = slice(b * BAND, (b + 1) * BAND)
                nc.vector.tensor_copy(out=xcat[0:D_IN, cs],
                                      in_=xh_b[b][0:D_IN, :])
                nc.vector.tensor_copy(out=xcat[D_IN:2 * D_IN, cs],
                                      in_=z2_b[b][0:D_IN, :])
            # u rows (64:128 of sig) into b-stacked u_st; b0 needs a
            # cross-quadrant move -> two 32-wide DVE copies
            nc.vector.tensor_copy(out=u_st[0:32, sl], in_=sig[0][64:96, :])
            nc.vector.tensor_copy(out=u_st[32:64, sl], in_=sig[0][96:128, :])
            nc.vector.tensor_copy(out=u_st[64:128, sl], in_=sig[1][64:128, :])
            for b in range(B_LOC):
                rows = slice(b * D_H, (b + 1) * D_H)
                cs = slice(b * BAND, (b + 1) * BAND)
                nc.tensor.matmul(pcx[rows, :], lhsT=wcxz_sb[:, :],
                                 rhs=xcat[:, cs], start=True, stop=False)
                nc.tensor.matmul(pcx[rows, :], lhsT=wcx1_sb[:, :],
                                 rhs=z1sl(g, b)[0:D_IN, :], start=False,
                                 stop=False)
                nc.tensor.matmul(pcx[rows, :], lhsT=wcrh_sb[0][0:D_H, :],
                                 rhs=rhsl(g, b), start=False, stop=True)
            nc.scalar.activation(c_x[:, sl], pcx[:, :], COPY)
            if g == 3 or g == NG - 1:
                for b in range(B_LOC):
                    nm_half(rhf8, rhdr, b, g // 4, D_H, BH)

        # stage-B diffusion groups split front (j2 0..7, z1-half-1 only) /
        # back (j2 8..13) / tail (j2 14,15 + psum copies): three fronts of
        # independent work queue ahead of the first half-2-dependent matmul
        # so the z1 half-2 transpose chain never stalls the in-order PE.
        def diffB_front(g):
            pss = [pspool.tile([C, BAND], F32, tag="ps", name=f"psb{b}")
                   for b in range(B_LOC)]
            for j2 in range(NJ2 // 2):
                for b in range(B_LOC):
                    nc.tensor.matmul(pss[b][:, :], lhsT=zt_st(z1dr, j2, b),
                                     rhs=adj_mv(g, j2), start=(j2 == 0),
                                     stop=False, perf_mode=DR)
            return pss

        def diffB_back(g, pss):
            for j2 in range(NJ2 // 2, NJ2 - 2):
                for b in range(B_LOC):
                    nc.tensor.matmul(pss[b][:, :], lhsT=zt_st(z1dr, j2, b),
                                     rhs=adj_mv(g, j2), start=False,
                                     stop=False, perf_mode=DR)

        def diffB_tail(g, pss):
            z2_b = [bpool.tile([C, BAND], F16, tag=f"z2b{b}", name=f"z2b{b}")
                    for b in range(B_LOC)]
            for b in range(B_LOC):
                for j2 in (NJ2 - 2, NJ2 - 1):
                    nc.tensor.matmul(pss[b][:, :], lhsT=zt_st(z1dr, j2, b),
                                     rhs=adj_mv(g, j2), start=False,
                                     stop=(j2 == NJ2 - 1), perf_mode=DR)
                nc.scalar.activation(z2_b[b][:, :], pss[b][:, :], COPY,
                                     scale=CB)
            return z2_b

        z2bands = [None, None]
        heads = [diffB_front(0), diffB_front(1), diffB_front(2)]
        diffB_back(0, heads[0])
        z2bands[0] = diffB_tail(0, heads[0])
        for g in range(NG):
            if g + 1 < NG:
                diffB_back(g + 1, heads[g + 1])
                z2bands[(g + 1) % 2] = diffB_tail(g + 1, heads[g + 1])
            conv_band(g, *lds[g])
            if g + 3 < NG:
                heads.append(diffB_front(g + 3))
            if g + 2 < NG:
                lds.append(loads(g + 2))

        # ================= stage C: zc1 = A rh ==================
        def consC(g, ps):
            sl = slice(g * BAND, (g + 1) * BAND)
            zc1_b = bpool.tile([BH, BAND], F8, tag="zc1f8", name="zc1b")
            nc.vector.tensor_scalar_mul(zc1_b[:, :], ps[:, :], CC)
            pc1 = pspool.tile([BH, BAND], F32, tag="ps", name="pc1")
            for b in range(B_LOC):
                rows = slice(b * D_H, (b + 1) * D_H)
                nc.tensor.matmul(pc1[rows, :], lhsT=wcrh_sb[1][rows, :],
                                 rhs=zc1_b[rows, :], start=True, stop=True)
            nc.vector.tensor_add(out=c_x[:, sl], in0=c_x[:, sl],
                                 in1=pc1[:, :])
            # one u16 transpose per band + DVE de-interleave into (j2, i, c)
            nm8 = bpool.tile([CH, 1536], F8, tag="nm8", name="nm8")
            nc.sync.dma_start(
                out=nm8[:, 0:512].bitcast(U16).rearrange(
                    "q (j c) -> q j c", j=2),
                in_=zc1_b[:, :].bitcast(U16), transpose=True)
            nc.vector.tensor_copy(
                out=zc1dr[:, 2 * g * 2 * CH:(2 * g + 2) * 2 * CH].rearrange(
                    "q (j i c) -> q j i c", j=2, i=2),
                in_=nm8[:, 0:512].rearrange("q (j c i) -> q j i c",
                                            j=2, i=2))

        psC = diff_cand(rhdr, 0, "psc")
        for g in range(NG):
            psN = diff_cand(rhdr, g + 1, "psc") if g + 1 < NG else None
            consC(g, psC)
            psC = psN

        # ====== stage D: zc2 = A zc1, fused tanh + GRU combine ======
        def consD(g, ps):
            sl = slice(g * BAND, (g + 1) * BAND)
            hD = bpool.tile([BH, BAND], F16, tag="xhb0", name="hD")
            nc.sync.dma_start(out=hD[:, :], in_=hbs_d[:, sl])
            zc2_b = bpool.tile([BH, BAND], F16, tag="xhb1", name="zc2b")
            nc.scalar.activation(zc2_b[:, :], ps[:, :], COPY, scale=CD)
            pc2 = pspool.tile([BH, BAND], F32, tag="ps", name="pc2")
            for b in range(B_LOC):
                rows = slice(b * D_H, (b + 1) * D_H)
                nc.tensor.matmul(pc2[rows, :], lhsT=wcrh_sb[2][rows, :],
                                 rhs=zc2_b[rows, :], start=True, stop=True)
            nc.vector.tensor_add(out=pc2[:, :], in0=pc2[:, :], in1=c_x[:, sl])
            cst = bpool.tile([BH, BAND], F16, tag="z2b0", name="cst")
            nc.scalar.activation(cst[:, :], pc2[:, :], TANH, bias=bc_sb[:, :])
            nc.vector.tensor_sub(out=hD[:, :], in0=hD[:, :], in1=cst[:, :])
            nc.vector.tensor_mul(out=hD[:, :], in0=u_st[:, sl], in1=hD[:, :])
            nc.vector.tensor_add(out=cst[:, :], in0=cst[:, :], in1=hD[:, :])
            w = min(BAND, NN - g * BAND)
            for b in range(B_LOC):
                nc.sync.dma_start(
                    out=out_d[b][:, g * BAND:g * BAND + w],
                    in_=cst[b * D_H:(b + 1) * D_H, 0:w])

        psD = diff_cand(zc1dr, 0, "psd")
        for g in range(NG):
            psN = diff_cand(zc1dr, g + 1, "psd") if g + 1 < NG else None
            consD(g, psD)
            psD = psN


# ---- host-side driver ----
_CACHED_NC = None
TRACE = False
TRACE_DIR = None
LAST_RESULTS = None


def _host_prep(x, h, adj, Wf, bf, Wu, bu, Wc, bc):
    A = np.zeros((NP, NP), dtype=np.float32)
    A[:NN, :NN] = adj * S_A
    # paired node order, i-major: at8[p, (g, j2, i, m)] = A[g*512+m,
    # j2*256+2p+i]
    at8 = np.ascontiguousarray(
        A.reshape(NG, BAND, NJ2, CH, 2).transpose(3, 0, 2, 4, 1)
        .reshape(CH, -1)).astype(NP8)

    WfT = Wf.T.astype(np.float32)     # [288, 64]
    WuT = Wu.T.astype(np.float32)
    WcT = Wc.T.astype(np.float32)
    wgk = []
    for k in range(3):
        m = np.concatenate([WfT[k * 96:(k + 1) * 96],
                            WuT[k * 96:(k + 1) * 96]], axis=1)
        if k == 1:
            m = m / S_Z1
        wgk.append(m)
    wg = np.ascontiguousarray(np.stack(wgk)).astype(np.float16)
    wcxz = np.ascontiguousarray(
        np.concatenate([WcT[0:32], WcT[192:224]], axis=0)).astype(np.float16)
    wcx1 = np.ascontiguousarray(WcT[96:128] / S_Z1).astype(np.float16)
    wcrh = np.ascontiguousarray(np.stack(
        [np.concatenate([WcT[32:96]] * B_LOC, axis=0),
         np.concatenate([WcT[128:192]] * B_LOC, axis=0) / S_ZC1,
         np.concatenate([WcT[224:288]] * B_LOC, axis=0)])).astype(np.float16)

    bfu = np.concatenate([bf, bu]).reshape(BH, 1).astype(np.float32)
    bc2 = np.concatenate([bc] * B_LOC).reshape(BH, 1).astype(np.float32)
    shared = {"at": at8, "wg": wg, "wcxz": wcxz, "wcx1": wcx1, "wcrh": wcrh,
              "bfu": bfu, "bcb": bc2}

    in_maps = []
    for core in range(NCORES):
        bs = slice(core * B_LOC, (core + 1) * B_LOC)
        zp = np.zeros((B_LOC, C, NP), dtype=np.float32)
        zp[:, :D_IN, :NN] = x[bs]
        zp[:, D_IN:, :NN] = h[bs]
        # paired, i-major: ztdr[p, (j2, i, b, c)] = S_Z * z[b, c, j2*256+2p+i]
        ztdr = np.ascontiguousarray(
            (zp.reshape(B_LOC, C, NJ2, CH, 2).transpose(3, 2, 4, 0, 1)
             .reshape(CH, -1)) * S_Z).astype(NP8)
        hbs = np.ascontiguousarray(
            zp[:, D_IN:, :].reshape(BH, NP)).astype(np.float16)
        in_maps.append(dict(shared, zt=ztdr, xh=zp.astype(np.float16),
                            hbs=hbs))
    return in_maps


def kernel(**inputs):
    global _CACHED_NC, LAST_RESULTS
    inputs = {k: np.asarray(v) for k, v in inputs.items()}
    if _CACHED_NC is None:
        _CACHED_NC = build_program()
    in_maps = _host_prep(**inputs)
    kw = {}
    if TRACE:
        kw = dict(trace=True, tmpdir=TRACE_DIR)
    res = run_bass_kernel_spmd(_CACHED_NC, in_maps,
                               core_ids=list(range(NCORES)), **kw)
    LAST_RESULTS = res
    outs = [res.results[i]["out"] for i in range(NCORES)]
    return np.concatenate(outs, axis=0).astype(np.float32)


if __name__ == "__main__":
    rng = np.random.default_rng(0)
    ins = {
        "x": rng.standard_normal((B, D_IN, NN), dtype=np.float32),
        "h": rng.standard_normal((B, D_H, NN), dtype=np.float32),
        "adj": rng.random((NN, NN), dtype=np.float32) / NN,
        "Wf": rng.standard_normal((D_H, 3 * C), dtype=np.float32) * 0.05,
        "Wu": rng.standard_normal((D_H, 3 * C), dtype=np.float32) * 0.05,
        "Wc": rng.standard_normal((D_H, 3 * C), dtype=np.float32) * 0.05,
        "bf": rng.standard_normal(D_H).astype(np.float32) * 0.05,
        "bu": rng.standard_normal(D_H).astype(np.float32) * 0.05,
        "bc": rng.standard_normal(D_H).astype(np.float32) * 0.05,
    }
    out = kernel(**ins)
    print(out.shape, out.dtype)
